# revision 32
# baseline (speedup 1.0000x reference)
"""Causal single-head attention on 8 TRN2 NeuronCores (v3).

Host staging (free w.r.t. HW exec time): x fed pre-transposed, chunk-
contiguous (8KB/partition lines per 512-seq chunk) in fp16; weights packed
as [Wq|Wk] and [Wv|Wk] fp16; biases packed into one [64, 3] f32 tensor.

Sharding (balanced ~39.4k PE col-ops/core):
  core i < 4  -> batch i,   q tiles {11..15}, kv 0:2048 (4 chunks)
  core i >= 4 -> batch i-4, q tiles {0..10},  kv 0:1408 (chunks 512/512/384)

Per-core pipeline:
  proj: x chunks DMA'd (first chunk split across gpsimd+scalar queues);
        [Wq|Wk] (q-chunks) or [Wv|Wk] stationary matmuls at N=w; A's chunk 2
        adds a partial Wq pass over tile 11's 128 cols only. kT evac on DVE
        (bias fused), V^T staged then PE-transposed into V1 [128, k, 65]
        (ones column -> free softmax row-sums), q evac'd via ScalarE + DMA to
        partitions 64:128.
  attn (scores TRANSPOSED, flash-style, k-outer): per k block scores^T =
        KT(k)-stationary x qT-moving, exp on ScalarE (scale=1/8) -> P^T fp16,
        diagonal blocks masked post-exp by 0/1 tri-mask (DVE), PV += V1(k) x
        P^T into per-region [65, W] PSUM accumulators (row 64 = denominators).
  finish per region (h-major, NO PE transposes): rinv = 1/pv[64] (DVE),
        bc = ones[1,64]^T x rinv (K=1 outer-product matmul), bc -> SBUF
        (GpSimd), out^T = pv[0:64] * bc (DVE) -> DMA [64, W] f32 rows.
        bv is folded into V during staging so out^T needs no bias add.
  Host transposes out^T -> [S, 64] per batch (free).
"""

import numpy as np

import concourse.bass as bass
import concourse.bacc as bacc
import concourse.mybir as mybir
from concourse.tile import TileContext
from concourse.masks import make_identity
from concourse.bass_utils import run_bass_kernel_spmd

B, S, D, H, P = 4, 2048, 1024, 64, 128
F32 = mybir.dt.float32
F16 = mybir.dt.float16
TILES_A = [11, 12, 13, 14, 15]
TILES_B = list(range(11))
KV_A, KV_B = 2048, 1408
NQ_MAX = 11

_nc_cache = {}


def _build():
    nc = bacc.Bacc(None, target_bir_lowering=False)
    # flat per-partition chunk-contiguous x^T: [p, sum_c 8*w_c] fp16
    xt_d = nc.dram_tensor("xt", [P, 16384], F16, kind="ExternalInput")
    wqk_d = nc.dram_tensor("wqk", [P, 8, P], F16, kind="ExternalInput")
    wkv_d = nc.dram_tensor("wkv", [P, 8, P], F16, kind="ExternalInput")
    bias_d = nc.dram_tensor("bias", [H, 3], F32, kind="ExternalInput")
    out_d = nc.dram_tensor("out", [H, NQ_MAX * P], F32, kind="ExternalOutput")

    with TileContext(nc) as tc, tc.tile_pool(name="const", bufs=1) as cpool:
        ident = cpool.tile([P, P], F16, tag="ident")
        nc.vector.memset(ident, 0.0)
        make_identity(nc, ident, nomemset=True)
        # 0/1 tri-mask in fp16: 1 where k <= q (keep), 0 above-diagonal
        trimask = cpool.tile([P, P], F16, tag="trimask")
        nc.vector.memset(trimask, 1.0)
        nc.gpsimd.affine_select(
            out=trimask, in_=trimask, compare_op=mybir.AluOpType.is_ge,
            fill=0.0, base=0, pattern=[[1, P]], channel_multiplier=-1,
        )
        # ones row on partition 64 (must match rinv's base partition in the
        # outer-product matmul)
        ones1_t = cpool.tile([H + 1, H], F16, tag="ones1_t")
        nc.vector.memset(ones1_t[H:H + 1, :], 1.0)
        ones1 = ones1_t[H:H + 1, :]
        wqk_sb = cpool.tile([P, 8, P], F16, tag="wqk_sb")
        nc.sync.dma_start(wqk_sb, wqk_d[:, :, :])
        wkv_sb = cpool.tile([P, 8, P], F16, tag="wkv_sb")
        nc.sync.dma_start(wkv_sb, wkv_d[:, :, :])
        bias_sb = cpool.tile([H, 3], F32, tag="bias_sb")
        nc.sync.dma_start(bias_sb, bias_d[:, :])
        bq_sb = bias_sb[:, 0:1]
        bk_sb = bias_sb[:, 1:2]
        bv_sb = bias_sb[:, 2:3]
        # warm the exp activation table during DMA wait
        zexp = cpool.tile([P, 1], F32, tag="zexp")
        nc.vector.memset(zexp, 0.0)
        nc.scalar.activation(zexp, zexp, mybir.ActivationFunctionType.Exp)

        def body(tiles, chunks, qchunks, regions, groups, sfx):
            """
            tiles: q tiles handled by this branch (ascending).
            chunks: list of (dram_off_elems, width) per kv chunk.
            qchunks: chunk idx -> (a, b) chunk-rel col range needing q evac,
                     'full' pass if (0, w), else partial Wq-only pass.
            regions: list of (col_lo, col_hi, pv_tag, pv_w, base, kstart,
                     kstop) PV accumulator regions over packed q cols.
            groups: ph2 schedule: list of (ks, tlo, thi).
            """
            nq = len(tiles)
            kv_len = sum(w for _, w in chunks)
            nkv = kv_len // P
            qw = nq * P

            def qcol(t):
                return tiles.index(t) * P

            with (
                tc.tile_pool(name="xp" + sfx, bufs=3) as xpool,
                tc.tile_pool(name="qk" + sfx, bufs=1) as qkpool,
                tc.tile_pool(name="vs" + sfx, bufs=2) as vspool,
                tc.tile_pool(name="ptp" + sfx, bufs=2) as ptpool,
                tc.tile_pool(name="rv" + sfx, bufs=2) as rvpool,
                tc.tile_pool(name="bcs" + sfx, bufs=2) as bcspool,
                tc.tile_pool(name="os" + sfx, bufs=1) as ospool,
                tc.tile_pool(name="prj" + sfx, bufs=2, space="PSUM") as prjpool,
                tc.tile_pool(name="scp" + sfx, bufs=3, space="PSUM") as scpool,
                tc.tile_pool(name="pvp" + sfx, bufs=1, space="PSUM") as pvpool,
                tc.tile_pool(name="kwp" + sfx, bufs=1, space="PSUM") as kwpool,
            ):
                qT = qkpool.tile([P, qw], F16, tag="qT")
                kT = qkpool.tile([P, kv_len], F16, tag="kT")
                v1 = qkpool.tile([P, nkv, H + 1], F16, tag="v1")
                nc.vector.memset(v1[:, :, H:H + 1], 1.0)
                ostage = ospool.tile([H, qw], F32, tag="ostage")

                pv_tiles = {}

                def alloc_pv(tag, w, pool_tag, nbufs):
                    pv = pvpool.tile([H + 1, w], F32, tag=pool_tag,
                                     bufs=nbufs, name="pv_" + tag)
                    pv_tiles[tag] = pv

                # Persistent never-read PSUM tile: warmup + keep-warm dummy
                # matmuls (pure PE FIFO filler, no semaphores) hold the HAM
                # activity window busy (idle de-boosts PE).
                warm = kwpool.tile([P, P], F32, tag="kw")
                for _w in range(20):
                    nc.tensor.matmul(
                        warm, ident, ident,
                        start=True, stop=True, skip_group_check=True,
                    )

                def keepwarm(n):
                    for _ in range(n):
                        nc.tensor.matmul(
                            warm[:, 0:H // 2], ident, ident[:, 0:H // 2],
                            start=True, stop=True, skip_group_check=True,
                        )

                def proj_chunk(c, split=False, eng=None):
                    off, w = chunks[c]
                    s0 = sum(ww for _, ww in chunks[:c])
                    x_t = xpool.tile([P, 8, 512], F16, tag="x")
                    src = xt_d[:, off:off + 8 * w].rearrange(
                        "p (j s) -> p j s", j=8)
                    if split:
                        # halves as two DMAs (SWDGE round-robins queues) so
                        # the first j-steps can start at half-chunk
                        nc.gpsimd.dma_start(x_t[:, 0:4, :w], src[:, 0:4, :])
                        nc.gpsimd.dma_start(x_t[:, 4:8, :w], src[:, 4:8, :])
                    else:
                        nc.gpsimd.dma_start(x_t[:, :, :w], src)
                    qr = qchunks.get(c)
                    full_q = qr is not None and qr == (0, w)
                    vstage = vspool.tile([P, 512], F16, tag="vstage")
                    if full_q:
                        # [Wq|Wk] stationary; separate Wv pass (M=64)
                        qk_ps = prjpool.tile([P, 512], F32, tag="mm")
                        for j in range(8):
                            nc.tensor.matmul(
                                qk_ps[:, :w], wqk_sb[:, j, :], x_t[:, j, :w],
                                start=(j == 0), stop=(j == 7),
                                skip_group_check=True,
                            )
                        v_ps = prjpool.tile([H, 512], F32, tag="mm")
                        for j in range(8):
                            nc.tensor.matmul(
                                v_ps[:, :w], wkv_sb[:, j, 0:H], x_t[:, j, :w],
                                start=(j == 0), stop=(j == 7),
                                skip_group_check=True,
                            )
                        ts = [t for t in tiles if s0 <= t * P < s0 + w]
                        a = ts[0] * P - s0
                        b = ts[-1] * P + P - s0
                        qtmp = vspool.tile([H, 512], F16, tag="qtmp")
                        nc.vector.tensor_scalar_add(
                            qtmp[:, :b - a], qk_ps[0:H, a:b], bq_sb
                        )
                        nc.sync.dma_start(
                            qT[H:P, qcol(ts[0]):qcol(ts[-1]) + P],
                            qtmp[:, :b - a],
                        )
                        nc.vector.tensor_scalar_add(
                            kT[H:P, s0:s0 + w], qk_ps[H:P, :w], bk_sb
                        )
                        nc.vector.tensor_scalar_add(
                            vstage[0:H, :w], v_ps[:, :w], bv_sb
                        )
                    else:
                        # [Wv|Wk] stationary: V rows 0:64, K rows 64:128
                        kv_ps = prjpool.tile([P, 512], F32, tag="mm")
                        for j in range(8):
                            nc.tensor.matmul(
                                kv_ps[:, :w], wkv_sb[:, j, :], x_t[:, j, :w],
                                start=(j == 0), stop=(j == 7),
                                skip_group_check=True,
                            )
                        if qr is not None:
                            # partial Wq-only pass over the q sub-range
                            a, b = qr
                            qp_ps = prjpool.tile([H, 512], F32, tag="mm")
                            for j in range(8):
                                nc.tensor.matmul(
                                    qp_ps[:, :b - a], wqk_sb[:, j, 0:H],
                                    x_t[:, j, a:b],
                                    start=(j == 0), stop=(j == 7),
                                    skip_group_check=True,
                                )
                            ts = [t for t in tiles
                                  if s0 + a <= t * P < s0 + b]
                            qtmp = vspool.tile([H, 512], F16, tag="qtmp")
                            nc.vector.tensor_scalar_add(
                                qtmp[:, :b - a], qp_ps[:, :b - a], bq_sb
                            )
                            nc.sync.dma_start(
                                qT[H:P, qcol(ts[0]):qcol(ts[-1]) + P],
                                qtmp[:, :b - a],
                            )
                        nc.vector.tensor_scalar_add(
                            kT[H:P, s0:s0 + w], kv_ps[H:P, :w], bk_sb
                        )
                        nc.vector.tensor_scalar_add(
                            vstage[0:H, :w], kv_ps[0:H, :w], bv_sb
                        )
                    # V^T -> V1 (seq-major) via PE transpose
                    ntile = w // P
                    vt_ps = prjpool.tile([P, 4 * H], F16, tag="mm")
                    for u in range(ntile):
                        nc.tensor.transpose(
                            vt_ps[:, u * H:(u + 1) * H],
                            vstage[0:H, u * P:(u + 1) * P],
                            ident[0:H, 0:H],
                        )
                    k0 = s0 // P
                    for u in range(ntile):
                        nc.vector.tensor_copy(
                            v1[:, k0 + u, 0:H], vt_ps[:, u * H:(u + 1) * H]
                        )

                def emit_pv(st):
                    # kstart/kstop are the first/last k for a region in
                    # EMISSION order (A emits k-groups high-to-low).
                    k, a, b, pt = st
                    for lo, hi, tag, _, base, kstart, kstop in regions:
                        s = max(a, lo)
                        e = min(b, hi)
                        if s >= e:
                            continue
                        pv = pv_tiles[tag]
                        nc.tensor.matmul(
                            pv[:, s - base:e - base], v1[:, k, :],
                            pt[:, s - a:e - a],
                            start=(k == kstart), stop=(k == kstop),
                            skip_group_check=True,
                        )

                def ph2_group(ks, tlo, thi, inject=None):
                    def emit_front(k):
                        ai = tlo
                        while tiles[ai] < k:
                            ai += 1
                        a, b = ai * P, thi * P
                        pt = ptpool.tile([P, qw], F16, tag="pt")
                        s = a
                        while s < b:
                            e = min((s // 512 + 1) * 512, b)
                            sc = scpool.tile([P, 512], F32, tag="sc")
                            nc.tensor.matmul(
                                sc[:, :e - s],
                                kT[H:P, k * P:(k + 1) * P],
                                qT[H:P, s:e],
                                start=True, stop=True, skip_group_check=True,
                            )
                            nc.scalar.activation(
                                pt[:, s - a:e - a], sc[:, :e - s],
                                mybir.ActivationFunctionType.Exp, scale=0.125,
                            )
                            s = e
                        if tiles[ai] == k:
                            nc.gpsimd.tensor_tensor(
                                pt[:, 0:P], pt[:, 0:P], trimask,
                                op=mybir.AluOpType.mult,
                            )
                        return (k, a, b, pt)

                    pend = []
                    for ki, k in enumerate(ks):
                        pend.append(emit_front(k))
                        keepwarm(1)
                        if len(pend) > 2:
                            emit_pv(pend.pop(0))
                        if ki == 1 and inject is not None:
                            # previous group's finish ops land here so the
                            # PE has score/PV work queued ahead of the
                            # finish outer-product (which waits on DVE)
                            inject()
                    for st in pend:
                        emit_pv(st)
                        keepwarm(1)
                    if len(ks) <= 2 and inject is not None:
                        inject()

                def finish_region(lo, hi, tag, base):
                    W = hi - lo
                    pv = pv_tiles[tag]
                    # rinv lives on partition 64 (same as pv's denominator
                    # row — engines cannot shift partition offsets)
                    rinv = rvpool.tile([H + 1, 512], F16, tag="rinv")
                    with nc.allow_low_precision("fp16 recip within 2e-2 tol"):
                        nc.vector.reciprocal(
                            rinv[H:H + 1, :W],
                            pv[H:H + 1, lo - base:hi - base])
                    bc = prjpool.tile([H, 512], F32, tag="mm")
                    nc.tensor.matmul(
                        bc[:, :W], ones1, rinv[H:H + 1, :W],
                        start=True, stop=True, skip_group_check=True,
                    )
                    bcs = bcspool.tile([H, 512], F16, tag="bcs")
                    nc.vector.tensor_copy(bcs[:, :W], bc[:, :W])
                    nc.vector.tensor_tensor(
                        ostage[:, lo:hi], pv[0:H, lo - base:hi - base],
                        bcs[:, :W], op=mybir.AluOpType.mult,
                    )
                    nc.sync.dma_start(out_d[:, lo:hi], ostage[:, lo:hi])

                if sfx == "a":
                    # chunk order 2,3,0,1 and k-groups {8-11} FIRST so every
                    # pv region issues exactly ONE start=True range per PSUM
                    # bank (a sub-range start on a bank with an open
                    # accumulation group wipes the rest of the bank).
                    alloc_pv("pvA", 512, "pvA", 1)
                    alloc_pv("pvB", 128, "pvB", 1)
                    proj_chunk(2, split=True)
                    proj_chunk(3, split=True)
                    ph2_group([8, 9, 10, 11], 0, nq)
                    proj_chunk(0)
                    ph2_group([12, 13, 14, 15], 0, nq)
                    proj_chunk(1)
                    ph2_group([0, 1, 2, 3], 0, nq)
                    ph2_group([4, 5, 6, 7], 0, nq)
                    finish_region(0, 512, "pvA", 0)
                    finish_region(512, 640, "pvB", 512)
                else:
                    proj_chunk(0, split=True)
                    proj_chunk(1)
                    alloc_pv("pv0", 512, "pv", 2)
                    ph2_group(list(range(4)), 0, 4)
                    proj_chunk(2)
                    alloc_pv("pv1", 512, "pv", 2)
                    ph2_group(list(range(8)), 4, 8,
                              inject=lambda: finish_region(0, 512, "pv0", 0))
                    alloc_pv("pv2", 384, "pv", 2)
                    ph2_group(list(range(11)), 8, nq,
                              inject=lambda: finish_region(512, 1024,
                                                           "pv1", 512))
                    finish_region(1024, 1408, "pv2", 1024)

            return None

        # B (the longer body) is the fall-through side: the TAKEN far branch
        # costs ~4us of sequencer fetch, so the shorter A side absorbs it.
        pid = nc.partition_id(engines=mybir.ALL_ENGINES)
        with tc.If(pid >= 4) as cmp:
            body(
                TILES_B,
                chunks=[(0, 512), (4096, 512), (8192, 384)],
                qchunks={0: (0, 512), 1: (0, 512), 2: (0, 384)},
                regions=[
                    (0, 512, "pv0", 512, 0, 0, 3),
                    (512, 1024, "pv1", 512, 512, 0, 7),
                    (1024, 1408, "pv2", 384, 1024, 0, 10),
                ],
                groups=None, sfx="b",
            )
        with cmp.Else():
            body(
                TILES_A,
                chunks=[(0, 512), (4096, 512), (8192, 512), (12288, 512)],
                qchunks={3: (0, 512), 2: (384, 512)},
                # packed q cols: tile 11 -> 0:128, 12..15 -> 128:640
                regions=[
                    (0, 512, "pvA", 512, 0, 8, 7),
                    (512, 640, "pvB", 128, 512, 8, 7),
                ],
                groups=None, sfx="a",
            )

    nc.finalize()
    return nc


def get_nc():
    if "nc" not in _nc_cache:
        _nc_cache["nc"] = _build()
    return _nc_cache["nc"]


def _install_ntff_hook():
    """Recreate the antenv.axon_hooks NTFF shim this image lacks (test-only)."""
    import sys, types
    try:
        import antenv.axon_hooks  # noqa
        return
    except ImportError:
        pass
    try:
        import antenv
        from trn_agent_boot.trn_boot import _ntff_profile_via_ctypes
        mod = types.ModuleType("antenv.axon_hooks")
        holder = {}
        mod.set_axon_ntff_profile_hook = lambda h: holder.__setitem__("h", h)
        mod.get_axon_ntff_profile_hook = lambda: holder.get("h")
        sys.modules["antenv.axon_hooks"] = mod
        antenv.axon_hooks = mod
        h = _ntff_profile_via_ctypes("/opt/axon/libaxon_pjrt.so")
        if h is not None:
            holder["h"] = h
    except Exception as e:  # profiling is best-effort
        print(f"ntff hook install failed: {e}")


def _stage_x(xt, chunks):
    """xt [1024, 2048] fp16 -> flat [128, 16384] chunk-contiguous."""
    out = np.zeros((P, 16384), dtype=np.float16)
    col = 0
    for s0, w in chunks:
        blk = xt[:, s0:s0 + w].reshape(8, P, w).transpose(1, 0, 2)
        out[:, col:col + 8 * w] = blk.reshape(P, 8 * w)
        col += 8 * w
    return out


def kernel(x, Wq, bq, Wk, bk, Wv, bv, _want_results=False, _trace=False):
    if _trace:
        _install_ntff_hook()
    x = np.asarray(x, dtype=np.float32)
    xts = [np.ascontiguousarray(x[b].T).astype(np.float16) for b in range(B)]
    xa = [_stage_x(t, [(0, 512), (512, 512), (1024, 512), (1536, 512)])
          for t in xts]
    xb = [_stage_x(t, [(0, 512), (512, 512), (1024, 384)]) for t in xts]

    def pack_w(a, b2):
        w = np.concatenate([np.asarray(a), np.asarray(b2)], axis=1)
        w = w.reshape(8, P, P).transpose(1, 0, 2)
        return np.ascontiguousarray(w).astype(np.float16)

    wqk = pack_w(Wq, Wk)
    wkv = pack_w(Wv, Wk)
    bias = np.stack([np.asarray(bq), np.asarray(bk), np.asarray(bv)],
                    axis=1).astype(np.float32)
    nc = get_nc()
    in_maps = []
    for core in range(8):
        b = core % 4
        in_maps.append({
            "xt": xa[b] if core < 4 else xb[b],
            "wqk": wqk, "wkv": wkv, "bias": bias,
        })
    res = run_bass_kernel_spmd(
        nc, in_maps, core_ids=list(range(8)), trace=_trace,
        **({"trace_cores": list(range(8))} if _trace else {}),
    )
    out = np.empty((B, S, H), dtype=np.float32)
    for core in range(8):
        b = core % 4
        tiles = TILES_A if core < 4 else TILES_B
        o = res.results[core]["out"]  # [64, 1408]
        for si, t in enumerate(tiles):
            out[b, t * P:(t + 1) * P, :] = o[:, si * P:(si + 1) * P].T
    if _want_results:
        return out, res
    return out


# revision 39
# speedup vs baseline: 1.1980x; 1.1980x over previous
"""Causal single-head attention on 8 TRN2 NeuronCores (v2).

Host staging (free w.r.t. HW exec time): x fed pre-transposed as x^T in
fp16 (halves DMA, kills all on-chip x transposes), weights packed as
[Wq|Wk] and [Wk|Wv] fp16, bv pre-broadcast to [128, 64].

Sharding: core i < 4  -> batch i,   q tiles {11..15}, kv 0:2048
          core i >= 4 -> batch i-4, q tiles {0..10},  kv 0:1408

Per-core pipeline:
  proj: xT chunks (512 rows) DMA'd d-major; [Wq|Wk] or [Wk|Wv] stationary
        matmuls at N=512 -> qT/kT (h-major, fp16, biases fused on
        ScalarE/DVE evac) and V^T staging -> PE-transpose -> V1 (seq-major
        [128, k, 65] with a ones column for free softmax row-sums).
  attn (scores computed TRANSPOSED, flash-style, k-outer):
        per k block: scores^T = KT(k)-stationary x qT-moving (one MM per
        512-col group), exp on ScalarE (scale=1/8) -> P^T fp16, diagonal
        block zeroed post-exp by a 0/1 tri-mask multiply (DVE, fp16 2x),
        PV += V1(k)-stationary x P^T-moving into a [65, nq*128] PSUM
        accumulator (row 64 = softmax denominators).
  finish per tile: PSUM->SBUF fp16, PE transpose -> [128, 65],
        out = pv * (1/rowsum) + bv, DMA out fp32.
"""

import numpy as np

import concourse.bass as bass
import concourse.bacc as bacc
import concourse.mybir as mybir
from concourse.tile import TileContext
from concourse.masks import make_identity
from concourse.bass_utils import run_bass_kernel_spmd

B, S, D, H, P = 4, 2048, 1024, 64, 128
F32 = mybir.dt.float32
F16 = mybir.dt.float16
TILES_A = [12, 13, 14, 15]
TILES_B = list(range(12))
KV_A, KV_B = 2048, 1536
NQ_MAX = 12

_nc_cache = {}


def _build():
    nc = bacc.Bacc(None, target_bir_lowering=False)
    # flat per-partition chunk-contiguous x^T: chunk c at cols [c*4096,
    # (c+1)*4096), 8KB contiguous per partition per chunk
    xt_d = nc.dram_tensor("xt", [P, 16384], F16, kind="ExternalInput")
    # weights host-prearranged to [p, dchunk, m] so DMA lines are 2KB
    wqk_d = nc.dram_tensor("wqk", [P, 8, P], F16, kind="ExternalInput")
    wkv_d = nc.dram_tensor("wkv", [P, 8, P], F16, kind="ExternalInput")
    bias_d = nc.dram_tensor("bias", [H, 3], F32, kind="ExternalInput")
    out_d = nc.dram_tensor("out", [NQ_MAX * P, H], F32, kind="ExternalOutput")

    with TileContext(nc) as tc, tc.tile_pool(name="const", bufs=1) as cpool:
        ident = cpool.tile([P, P], F16, tag="ident")
        nc.vector.memset(ident, 0.0)
        make_identity(nc, ident, nomemset=True)
        # 0/1 tri-mask in fp16: 1 where k <= q (keep), 0 above-diagonal
        trimask = cpool.tile([P, P], F16, tag="trimask")
        nc.vector.memset(trimask, 1.0)
        # keep (1.0) where y - x >= 0, i.e. k <= q; zero above the diagonal
        nc.gpsimd.affine_select(
            out=trimask, in_=trimask, compare_op=mybir.AluOpType.is_ge,
            fill=0.0, base=0, pattern=[[1, P]], channel_multiplier=-1,
        )
        wqk_sb = cpool.tile([P, 8, P], F16, tag="wqk_sb")
        nc.sync.dma_start(wqk_sb, wqk_d[:, :, :])
        wkv_sb = cpool.tile([P, 8, P], F16, tag="wkv_sb")
        nc.sync.dma_start(wkv_sb, wkv_d[:, :, :])
        bias_sb = cpool.tile([H, 3], F32, tag="bias_sb")
        nc.sync.dma_start(bias_sb, bias_d[:, :])
        bq_sb = bias_sb[:, 0:1]
        bk_sb = bias_sb[:, 1:2]
        bv_sb = bias_sb[:, 2:3]
        # warm the exp activation table during DMA wait
        zexp = cpool.tile([P, 1], F32, tag="zexp")
        nc.vector.memset(zexp, 0.0)
        nc.scalar.activation(zexp, zexp, mybir.ActivationFunctionType.Exp)

        def body(tiles, kv_len, sc_bufs, sfx):
            nq = len(tiles)
            nkv = kv_len // P
            qw = nq * P  # packed qT width
            # chunk boundaries in seq: chunk c covers rows [c*512, min((c+1)*512, kv_len))
            nck = (kv_len + 511) // 512

            def qcol(t):
                return tiles.index(t) * P

            with (
                tc.tile_pool(name="xp" + sfx, bufs=3) as xpool,
                tc.tile_pool(name="qk" + sfx, bufs=1) as qkpool,
                tc.tile_pool(name="vs" + sfx, bufs=2) as vspool,
                tc.tile_pool(name="ptp" + sfx, bufs=2) as ptpool,
                tc.tile_pool(name="pvs" + sfx, bufs=2) as pvspool,
                tc.tile_pool(name="rv" + sfx, bufs=2) as rvpool,
                tc.tile_pool(name="os" + sfx, bufs=1) as ospool,
                tc.tile_pool(name="prj" + sfx, bufs=2, space="PSUM") as prjpool,
                tc.tile_pool(name="scp" + sfx, bufs=sc_bufs, space="PSUM") as scpool,
                tc.tile_pool(name="pvp" + sfx, bufs=1, space="PSUM") as pvpool,
                tc.tile_pool(name="kwp" + sfx, bufs=1, space="PSUM") as kwpool,
            ):
                # qT/kT live on partitions 64:128 (K lands there from the
                # packed matmuls; Q is DMA'd across from its 0:64 evac)
                qT = qkpool.tile([P, qw], F16, tag="qT")
                kT = qkpool.tile([P, kv_len], F16, tag="kT")
                v1 = qkpool.tile([P, nkv, H + 1], F16, tag="v1")
                nc.vector.memset(v1[:, :, H:H + 1], 1.0)
                ostage = ospool.tile([P, nq, H], F32, tag="ostage")

                # Persistent never-read PSUM tile: warmup + keep-warm dummy
                # matmuls write here with no allocs, so they are pure PE
                # FIFO filler (no semaphores) that holds the HAM activity
                # window busy (idle or transposes de-boost PE to 1.2 GHz).
                warm = kwpool.tile([P, P], F32, tag="kw")
                for _w in range(44):
                    nc.tensor.matmul(
                        warm, ident, ident,
                        start=True, stop=True, skip_group_check=True,
                    )

                def keepwarm(n):
                    for _ in range(n):
                        nc.tensor.matmul(
                            warm[:, 0:H // 2], ident, ident[:, 0:H // 2],
                            start=True, stop=True, skip_group_check=True,
                        )


                def proj_chunk(c, with_q, split=False):
                    s0 = c * 512
                    w = min(512, kv_len - s0)
                    x_t = xpool.tile([P, 8, 512], F16, tag="x")
                    # SWDGE chunk DMAs; flat staging gives 8KB contiguous
                    # per-partition lines per chunk. First chunk split in
                    # d-halves so proj starts earlier.
                    src = xt_d[:, c * 4096:(c + 1) * 4096].rearrange(
                        "p (j s) -> p j s", j=8)
                    if split:
                        nc.gpsimd.dma_start(x_t[:, 0:4, :w], src[:, 0:4, :w])
                        nc.gpsimd.dma_start(x_t[:, 4:8, :w], src[:, 4:8, :w])
                    else:
                        nc.gpsimd.dma_start(x_t[:, :, :w], src[:, :, :w])
                    vstage = vspool.tile([P, 512], F16, tag="vstage")
                    if with_q:
                        # [Wq|Wk] stationary; separate Wv pass (M=64)
                        qk_ps = prjpool.tile([P, 512], F32, tag="mm")
                        for j in range(8):
                            nc.tensor.matmul(
                                qk_ps[:, :w], wqk_sb[:, j, :], x_t[:, j, :w],
                                start=(j == 0), stop=(j == 7),
                                skip_group_check=True,
                            )
                        v_ps = prjpool.tile([H, 512], F32, tag="mm")
                        for j in range(8):
                            nc.tensor.matmul(
                                v_ps[:, :w], wkv_sb[:, j, 0:H], x_t[:, j, :w],
                                start=(j == 0), stop=(j == 7),
                                skip_group_check=True,
                            )
                        # q evac (psum rows 0:64) then SBUF->SBUF DMA up to
                        # partitions 64:128 where the scores matmuls want it
                        ts = [t for t in tiles if s0 <= t * P < s0 + w]
                        if ts:
                            a = ts[0] * P - s0
                            b = ts[-1] * P + P - s0
                            qtmp = vspool.tile([H, 512], F16, tag="qtmp")
                            nc.scalar.activation(
                                qtmp[:, :b - a], qk_ps[0:H, a:b],
                                mybir.ActivationFunctionType.Identity,
                                bias=bq_sb,
                            )
                            nc.sync.dma_start(
                                qT[H:P, qcol(ts[0]):qcol(ts[-1]) + P],
                                qtmp[:, :b - a],
                            )
                        nc.vector.tensor_scalar_add(
                            kT[H:P, s0:s0 + w], qk_ps[H:P, :w], bk_sb
                        )
                        nc.scalar.activation(
                            vstage[0:H, :w], v_ps[:, :w],
                            mybir.ActivationFunctionType.Identity,
                            bias=bv_sb,
                        )
                    else:
                        # [Wv|Wk] stationary: V rows 0:64, K rows 64:128
                        kv_ps = prjpool.tile([P, 512], F32, tag="mm")
                        for j in range(8):
                            nc.tensor.matmul(
                                kv_ps[:, :w], wkv_sb[:, j, :], x_t[:, j, :w],
                                start=(j == 0), stop=(j == 7),
                                skip_group_check=True,
                            )
                        nc.vector.tensor_scalar_add(
                            kT[H:P, s0:s0 + w], kv_ps[H:P, :w], bk_sb
                        )
                        nc.scalar.activation(
                            vstage[0:H, :w], kv_ps[0:H, :w],
                            mybir.ActivationFunctionType.Identity,
                            bias=bv_sb,
                        )
                    # V^T -> V1 (seq-major) via PE transpose
                    ntile = w // P
                    vt_ps = prjpool.tile([P, 4 * H], F16, tag="mm")
                    for u in range(ntile):
                        nc.tensor.transpose(
                            vt_ps[:, u * H:(u + 1) * H],
                            vstage[0:H, u * P:(u + 1) * P],
                            ident[0:H, 0:H],
                        )
                    k0 = s0 // P
                    for u in range(ntile):
                        nc.vector.tensor_copy(
                            v1[:, k0 + u, 0:H], vt_ps[:, u * H:(u + 1) * H]
                        )
                    return x_t

                def ph2_group(ks, tlo, thi, pv_ap, pv_base, kstart, kstop):
                    # blocks (k, t) for k in ks, t in tiles[tlo:thi] with
                    # t >= k; only emitted once both k's and t's chunks are
                    # projected. Software-pipelined: scores+exp run 2 k's
                    # ahead of the PV matmuls so the PE never waits on exp.
                    def emit_front(k):
                        ai = tlo
                        while tiles[ai] < k:
                            ai += 1
                        a, b = ai * P, thi * P
                        pt = ptpool.tile([P, b - a], F16, tag="pt")
                        s = a
                        while s < b:
                            e = min((s // 512 + 1) * 512, b)
                            sc = scpool.tile([P, 512], F32, tag="sc")
                            nc.tensor.matmul(
                                sc[:, :e - s],
                                kT[H:P, k * P:(k + 1) * P],
                                qT[H:P, s:e],
                                start=True, stop=True, skip_group_check=True,
                            )
                            nc.scalar.activation(
                                pt[:, s - a:e - a], sc[:, :e - s],
                                mybir.ActivationFunctionType.Exp, scale=0.125,
                            )
                            s = e
                        if tiles[ai] == k:
                            nc.vector.tensor_tensor(
                                pt[:, 0:P], pt[:, 0:P], trimask,
                                op=mybir.AluOpType.mult,
                            )
                        return (k, a, b, pt)

                    def emit_pv(st):
                        k, a, b, pt = st
                        s = a
                        while s < b:
                            e = min((s // 512 + 1) * 512, b)
                            nc.tensor.matmul(
                                pv_ap[:, s - pv_base:e - pv_base],
                                v1[:, k, :], pt[:, s - a:e - a],
                                start=(k == kstart), stop=(k == kstop),
                                skip_group_check=True,
                            )
                            s = e

                    pend = []
                    for k in ks:
                        pend.append(emit_front(k))
                        keepwarm(2)
                        if len(pend) > 2:
                            emit_pv(pend.pop(0))
                    for st in pend:
                        emit_pv(st)
                        keepwarm(1)

                def finish_tiles(tlist, pv_ap, pv_base):
                    # pipelined: the DVE copy for tile i+1 is emitted before
                    # tile i's transpose-dependent ops so DVE never bubbles
                    def do_copy(t):
                        i = tiles.index(t)
                        pvsb = pvspool.tile([H + 1, P], F16, tag="pvsb")
                        nc.vector.tensor_copy(
                            pvsb, pv_ap[:, i * P - pv_base:i * P - pv_base + P]
                        )
                        return (i, pvsb)
                    def do_rest(st):
                        i, pvsb = st
                        fin = prjpool.tile([P, H + 1], F16, tag="mm")
                        nc.tensor.transpose(fin, pvsb, ident[0:H + 1, 0:H + 1])
                        rinv = rvpool.tile([P, 1], F32, tag="rinv")
                        nc.vector.reciprocal(rinv, fin[:, H:H + 1])
                        nc.vector.tensor_scalar_mul(
                            ostage[:, i, :], fin[:, 0:H], rinv
                        )
                    stage = []
                    for t in tlist:
                        stage.append(do_copy(t))
                        if len(stage) > 1:
                            do_rest(stage.pop(0))
                    for st in stage:
                        do_rest(st)

                def dma_out(i0, i1):
                    nc.sync.dma_start(
                        out_d[i0 * P:i1 * P, :].rearrange("(i p) h -> p i h", p=P),
                        ostage[:, i0:i1, :],
                    )

                if sfx == "a":
                    # chunk order 3,2,0,1; one persistent 1-bank pv
                    # accumulator (all groups hit tiles 12-15)
                    pv = pvpool.tile([H + 1, qw], F32, tag="pv")
                    proj_chunk(3, with_q=True, split=True)
                    proj_chunk(2, with_q=False)
                    ph2_group([12, 13, 14, 15], 0, nq, pv, 0, 12, None)
                    proj_chunk(0, with_q=False)
                    ph2_group([8, 9, 10, 11], 0, nq, pv, 0, None, None)
                    proj_chunk(1, with_q=False)
                    ph2_group([0, 1, 2, 3], 0, nq, pv, 0, None, None)
                    ph2_group([4, 5, 6, 7], 0, nq, pv, 0, None, 7)
                    finish_tiles((12, 13), pv, 0)
                    dma_out(0, 2)
                    finish_tiles((14, 15), pv, 0)
                    dma_out(2, nq)
                else:
                    # per chunk c: t in chunk-c tiles, k = 0..max(t);
                    # disjoint t-ranges -> rotating per-group pv tiles
                    proj_chunk(0, with_q=True, split=True)
                    proj_chunk(1, with_q=True)
                    pv0 = pvpool.tile([H + 1, 512], F32, tag="pv", bufs=2)
                    ph2_group(list(range(4)), 0, 4, pv0, 0, 0, 3)
                    proj_chunk(2, with_q=True)
                    pv1 = pvpool.tile([H + 1, 512], F32, tag="pv", bufs=2)
                    ph2_group(list(range(8)), 4, 8, pv1, 512, 0, 7)
                    finish_tiles((0, 1, 2, 3), pv0, 0)
                    dma_out(0, 4)
                    pv2 = pvpool.tile([H + 1, 512], F32, tag="pv", bufs=2)
                    ph2_group(list(range(12)), 8, nq, pv2, 1024, 0, 11)
                    finish_tiles((4, 5, 6, 7), pv1, 512)
                    dma_out(4, 8)
                    finish_tiles((8, 9, 10, 11), pv2, 1024)
                    dma_out(8, nq)

            return None

        # B (the longer body) is the fall-through side: the TAKEN far branch
        # costs ~4us of sequencer fetch, so the shorter A side absorbs it.
        pid = nc.partition_id(engines=mybir.ALL_ENGINES)
        with tc.If(pid >= 4) as cmp:
            body(TILES_B, KV_B, 3, "b")
        with cmp.Else():
            body(TILES_A, KV_A, 4, "a")

    nc.finalize()
    return nc


def get_nc():
    if "nc" not in _nc_cache:
        _nc_cache["nc"] = _build()
    return _nc_cache["nc"]


def _install_ntff_hook():
    """Recreate the antenv.axon_hooks NTFF shim this image lacks (test-only)."""
    import sys, types
    try:
        import antenv.axon_hooks  # noqa
        return
    except ImportError:
        pass
    try:
        import antenv
        from trn_agent_boot.trn_boot import _ntff_profile_via_ctypes
        mod = types.ModuleType("antenv.axon_hooks")
        holder = {}
        mod.set_axon_ntff_profile_hook = lambda h: holder.__setitem__("h", h)
        mod.get_axon_ntff_profile_hook = lambda: holder.get("h")
        sys.modules["antenv.axon_hooks"] = mod
        antenv.axon_hooks = mod
        h = _ntff_profile_via_ctypes("/opt/axon/libaxon_pjrt.so")
        if h is not None:
            holder["h"] = h
    except Exception as e:  # profiling is best-effort
        print(f"ntff hook install failed: {e}")


def kernel(x, Wq, bq, Wk, bk, Wv, bv, _want_results=False, _trace=False):
    if _trace:
        _install_ntff_hook()
    x = np.asarray(x, dtype=np.float32)
    # flat chunk-contiguous staging: xt_flat[p, c*4096 + j*512 + s]
    # = x[b].T[j*128 + p, c*512 + s]
    xt = []
    for b in range(B):
        t = np.ascontiguousarray(x[b].T).astype(np.float16)
        t = t.reshape(8, P, 4, 512).transpose(1, 2, 0, 3)
        xt.append(np.ascontiguousarray(t.reshape(P, 16384)))

    def pack_w(a, b):
        # [D, 128] -> [p, dchunk, m] so each DMA partition line is 2KB
        w = np.concatenate([np.asarray(a), np.asarray(b)], axis=1)
        w = w.reshape(8, P, P).transpose(1, 0, 2)
        return np.ascontiguousarray(w).astype(np.float16)

    wqk = pack_w(Wq, Wk)
    wkv = pack_w(Wv, Wk)
    bias = np.stack([np.asarray(bq), np.asarray(bk), np.asarray(bv)],
                    axis=1).astype(np.float32)
    nc = get_nc()
    in_maps = []
    for core in range(8):
        b = core % 4
        in_maps.append({
            "xt": xt[b], "wqk": wqk, "wkv": wkv, "bias": bias,
        })
    res = run_bass_kernel_spmd(
        nc, in_maps, core_ids=list(range(8)), trace=_trace,
        **({"trace_cores": list(range(8))} if _trace else {}),
    )
    out = np.empty((B, S, H), dtype=np.float32)
    for core in range(8):
        b = core % 4
        tiles = TILES_A if core < 4 else TILES_B
        o = res.results[core]["out"][:len(tiles) * P].reshape(len(tiles), P, H)
        for si, t in enumerate(tiles):
            out[b, t * P:(t + 1) * P, :] = o[si]
    if _want_results:
        return out, res
    return out

